# revision 57
# baseline (speedup 1.0000x reference)
"""Trainium2 Bass kernel for CustomMultiHeadAttention (RoPE + causal MHA).

Sharding: 8 cores = 2 batches x 4 head-groups (4 heads each).
Each core computes, for its (batch, head-group):
  QT/KT = (Wq|Wk col-slice, half-permuted).T @ xT   -> [256, S] feature-major
  RoPE on QT/KT (full-tile DVE ops thanks to half-grouped layout)
  V = xT.T @ Wv col-slice                            -> [S, 256] natural
  per head: scoresT[k,q] = KT_h.T @ QT_h (row-packed over heads, K=64)
            expT = exp(scoresT/8) (no max subtraction; scores are bounded)
            causal: skip blocks above diagonal, 0/1-mask diagonal blocks
            ctxT[d,q] (+denominator row via ones-column in V_aug) = V_aug.T @ expT
  normalize ctxT by exp(-ln(den)) (both on ACT engine; DVE mult)
  partial_out = ctxT.T @ Wo row-slice               -> [S, 1024]
Host: sums the 4 head-group partials per batch, adds bo.

Perf notes (vs the first working version, 280us -> ~203us):
- score matmuls are full 128x128-mode with zero-padded K halves (KTe/KTo)
  instead of 64x128 tile_position matmuls: mixing tiling modes forces a PE
  array drain at EVERY mode switch, which re-throttled the HAM clock gate
  and held the whole kernel at 1.2 GHz (this alone was ~45us)
- all projection operands in bf16 (half the DMA bytes, FWL weight loads)
- projection/output matmuls are interleaved as filler units between the
  attention ki iterations: the ki loop is exp(ACT)-bound, and the PE
  instruction stream is in-order, so serial projection blocks would idle
  the PE mid-loop and the ACT engine during projections
- 1/den via ACT Ln+Exp instead of DVE reciprocal (8 cyc/elem iterative
  divide); both functions are forced into the one activation table set
  that holds ln AND exp so the ACT engine loads its table once
- warm-up matmuls on the mask tile during the initial DMAs so the PE's
  HAM clock-gate opens before real work arrives
"""

import os
import sys

for _p in ("/opt/trn_rl_repo", "/root/.axon_site/_ro/trn_rl_repo"):
    if os.path.isdir(_p) and _p not in sys.path:
        sys.path.insert(0, _p)

import numpy as np
import ml_dtypes

import concourse.bass as bass
import concourse.bacc as bacc
import concourse.mybir as mybir
import concourse.tile as tile
from concourse.bass_utils import run_bass_kernel_spmd

F32 = mybir.dt.float32
BF16 = mybir.dt.bfloat16
AF = mybir.ActivationFunctionType
ALU = mybir.AluOpType

NUM_HEADS = 16
HD = 64
D = NUM_HEADS * HD  # 1024
B = 2
S = 2048
NCORES = 8
HPC = 4            # heads per core
JC = HPC * HD      # 256 per-core projection width
P = 128

N_WARMUP = int(os.environ.get("KERNEL_WARMUP", "32"))


def build_core(tc, io, s_len=S):
    """Emit the per-core program. io: dict of DRAM APs. s_len: sequence length
    (parameterized so the simulator self-test can run a smaller size)."""
    nc = tc.nc
    SL = s_len
    NST = SL // P          # 128-row seq tiles
    NQC = SL // 512        # 512-wide q chunks
    NDT = D // P           # 8 k-tiles over d_model
    scale = 1.0 / np.sqrt(HD)

    xT_d, wq_d, wk_d, wv_d, wo_d = io["xT"], io["wq"], io["wk"], io["wv"], io["wo"]
    sin_d, cos_d, mask_d, out_d = io["sin"], io["cos"], io["mask"], io["out"]

    import contextlib
    with contextlib.ExitStack() as ctx:
        cpool = ctx.enter_context(tc.tile_pool(name="const", bufs=1))
        epool = ctx.enter_context(tc.tile_pool(name="expt", bufs=6))
        tpool = ctx.enter_context(tc.tile_pool(name="tmps", bufs=6))
        ps_scores = ctx.enter_context(tc.tile_pool(name="ps_sc", bufs=3, space="PSUM"))
        ps_ctx = ctx.enter_context(tc.tile_pool(name="ps_ctx", bufs=3, space="PSUM"))
        ps_fill = ctx.enter_context(tc.tile_pool(name="ps_fl", bufs=2, space="PSUM"))

        # ---- persistent SBUF tensors ----
        xT = cpool.tile([P, NDT, SL], BF16, tag="xT")
        wq = cpool.tile([P, NDT, JC], BF16, tag="wq")
        wk = cpool.tile([P, NDT, JC], BF16, tag="wk")
        wv = cpool.tile([P, NDT, JC], BF16, tag="wv")
        wo = cpool.tile([P, 2, D], BF16, tag="wo")
        sinf = cpool.tile([P, SL], BF16, tag="sinf")
        cosf = cpool.tile([P, SL], BF16, tag="cosf")
        mask = cpool.tile([P, P], BF16, tag="mask")
        QT = cpool.tile([P, 2, SL], BF16, tag="QT")     # plane 0: first halves
        KT = cpool.tile([P, 2, SL], BF16, tag="KT")
        # head-contiguous copies: plane p holds heads 2p (parts 0-63) and
        # 2p+1 (parts 64-127), dims 0-31 = rotated first half, 32-63 second.
        # K is split into zero-padded even/odd-head tiles so the score
        # matmuls run as full 128x128-mode (K=128 with a zero half) --
        # mixing 64x128-mode matmuls into the stream forces a PE array
        # drain at every tiling-mode switch.
        QTc = cpool.tile([P, 2, SL], BF16, tag="QTc")
        KTe = cpool.tile([P, 2, SL], BF16, tag="KTe")
        KTo = cpool.tile([P, 2, SL], BF16, tag="KTo")
        V = cpool.tile([P, NST, HPC, 2 * HD], BF16, tag="V")
        ctxA = cpool.tile([P, SL], BF16, tag="ctxA")     # heads 0,1 (j on partitions)
        ctxB = cpool.tile([P, SL], BF16, tag="ctxB")     # heads 2,3

        # ---- input DMAs (weights first; xT split per s-chunk) ----
        def xt_load(qc_i):
            for dt_i in range(NDT):
                nc.sync.dma_start(
                    xT[:, dt_i, qc_i * 512:(qc_i + 1) * 512],
                    xT_d[dt_i * P:(dt_i + 1) * P, qc_i * 512:(qc_i + 1) * 512])

        # mask first: the warm-up matmuls below only need it
        nc.sync.dma_start(mask[:], mask_d[:])
        # HAM warm-up: ~24 cheap matmuls on the mask keep the PE busy while
        # the real inputs stream in, so the clock gate opens before work starts
        for _ in range(N_WARMUP):
            wps = ps_fill.tile([P, 512], F32, tag="fill", name="warm")
            nc.tensor.matmul(wps[:, :P], lhsT=mask[:], rhs=mask[:],
                             start=True, stop=True)

        # per-ktile weight loads so the first projection matmul can start
        # as soon as its first 64KB slice lands rather than after the full
        # weight tensor
        def w_load(w_sb, w_d):
            for dt_i in range(NDT):
                nc.sync.dma_start(w_sb[:, dt_i, :],
                                  w_d[dt_i * P:(dt_i + 1) * P, :])

        def sc_load(qc_i):
            sl = slice(qc_i * 512, (qc_i + 1) * 512)
            nc.sync.dma_start(sinf[:, sl], sin_d[:, sl])
            nc.sync.dma_start(cosf[:, sl], cos_d[:, sl])

        w_load(wq, wq_d)
        xt_load(0)
        sc_load(0)
        w_load(wk, wk_d)
        w_load(wv, wv_d)
        xt_load(1)
        for qc_i in range(1, NQC):
            sc_load(qc_i)
        nc.gpsimd.memset(V[:, :, :, HD:], 1.0)  # denominator ones columns
        nc.gpsimd.memset(KTe[64:128, :, :], 0.0)  # zero halves, written once
        nc.gpsimd.memset(KTo[0:64, :, :], 0.0)

        def rope(pA, pB, out_sb, qc):
            """RoPE rotate the two projection psum tiles into QT/KT (DVE),
            then rearrange into the head-contiguous QTc/KTc copy (DMA)."""
            sl = slice(qc * 512, qc * 512 + 512)
            csl, ssl = cosf[:, sl], sinf[:, sl]
            t1 = tpool.tile([P, 512], F32, tag="t1")
            t2 = tpool.tile([P, 512], F32, tag="t2")
            nc.vector.tensor_tensor(t1[:], pA[:], csl, ALU.mult)
            nc.vector.tensor_tensor(t2[:], pB[:], ssl, ALU.mult)
            nc.vector.tensor_tensor(out_sb[:, 0, sl], t1[:], t2[:], ALU.subtract)
            t3 = tpool.tile([P, 512], F32, tag="t1")
            t4 = tpool.tile([P, 512], F32, tag="t2")
            nc.vector.tensor_tensor(t3[:], pA[:], ssl, ALU.mult)
            nc.vector.tensor_tensor(t4[:], pB[:], csl, ALU.mult)
            nc.vector.tensor_tensor(out_sb[:, 1, sl], t3[:], t4[:], ALU.add)
            for h in range(HPC):
                if out_sb is QT:
                    dst = QTc
                else:
                    dst = KTe if h % 2 == 0 else KTo
                for half in range(2):
                    nc.sync.dma_start(
                        dst[64 * (h % 2) + 32 * half:64 * (h % 2) + 32 * half + 32,
                            h // 2, sl],
                        out_sb[32 * h:32 * h + 32, half, sl])

        def qk_units(qc, w_sb, out_sb):
            """qk projection of one s-chunk as two filler units (jt halves)."""
            sl = slice(qc * 512, qc * 512 + 512)
            state = {}

            def mm_half(jt):
                pp = ps_fill.tile([P, 512], F32, tag="fill", name=f"qk{jt}")
                for dt_i in range(NDT):
                    nc.tensor.matmul(
                        pp[:],
                        lhsT=w_sb[:, dt_i, jt * P:(jt + 1) * P],
                        rhs=xT[:, dt_i, sl],
                        start=(dt_i == 0), stop=(dt_i == NDT - 1),
                    )
                return pp

            def uA():
                state["pA"] = mm_half(0)

            def uB():
                pB = mm_half(1)
                rope(state["pA"], pB, out_sb, qc)

            return [uA, uB]

        def v_unit(st):
            """Project one 128-row seq tile of V into V_sb (strided, +1 col)."""
            def u():
                ppf = ps_fill.tile([P, 512], F32, tag="fill", name="vproj")
                pp = ppf[:, :JC]
                for dt_i in range(NDT):
                    nc.tensor.matmul(
                        pp[:],
                        lhsT=xT[:, dt_i, st * P:(st + 1) * P],
                        rhs=wv[:, dt_i, :],
                        start=(dt_i == 0), stop=(dt_i == NDT - 1),
                    )
                nc.vector.tensor_copy(
                    out=V[:, st, :, 0:HD],
                    in_=pp[:].rearrange("p (h d) -> p h d", h=HPC),
                )
            return u

        def out_units(qc, evac="vector"):
            """Output projection of one q-chunk as 8 filler units (st, nh).
            evac picks the PSUM-evacuation engine: the DVE mid-kernel (ACT
            is exp-saturated there), the ACT engine for the final chunk's
            tail (the exps are done and Copy shares the loaded table)."""
            units = []
            for st in range(4 * qc, 4 * qc + 4):
                for nh in range(2):
                    def u(st=st, nh=nh):
                        pp = ps_fill.tile([P, 512], F32, tag="fill", name="oproj")
                        for jt, csb in enumerate((ctxA, ctxB)):
                            nc.tensor.matmul(
                                pp[:],
                                lhsT=csb[:, st * P:(st + 1) * P],
                                rhs=wo[:, jt, nh * 512:nh * 512 + 512],
                                start=(jt == 0), stop=(jt == 1),
                            )
                        ot = tpool.tile([P, 512], BF16, tag="ostage")
                        if evac == "scalar":
                            nc.scalar.activation(ot[:], pp[:], AF.Copy)
                        else:
                            nc.vector.tensor_copy(out=ot[:], in_=pp[:])
                        nc.sync.dma_start(
                            out_d[st * P:(st + 1) * P, nh * 512:nh * 512 + 512],
                            ot[:])
                    units.append(u)
            return units

        def attention_pair(qc, pair, fillers, start_frac=0.0):
            """Causal attention ki-loop for one q-chunk and one head pair.
            `fillers` are projection units interleaved between ki iterations
            so the PE stays busy while the ACT engine works through the exps
            (the attention loop is exp-bound). Units start after
            `start_frac` of the ki iterations so their dependencies (e.g.
            the previous chunk's normalize) have time to resolve. Returns
            the two ctx psum tiles (normalization happens in
            normalize_pair)."""
            heads = (2 * pair, 2 * pair + 1)
            n_ki = 4 * qc + 4
            ki0 = int(start_frac * n_ki)
            ctx_ps = [ps_ctx.tile([P, 512], F32, tag="ctx", name=f"ctx{pair}{i}")
                      for i in range(len(heads))]
            emitted = 0
            nf = len(fillers)
            for ki in range(n_ki):
                diag_r = ki - 4 * qc
                c0 = 128 * diag_r if diag_r >= 0 else 0
                nv = 512 - c0
                qsl = slice(qc * 512 + c0, qc * 512 + 512)
                st_ps = [ps_scores.tile([P, 512], F32, tag="sc", name=f"sc{i}")
                         for i in range(len(heads))]
                for hh, h in enumerate(heads):
                    nc.tensor.matmul(
                        st_ps[hh][:, :nv],
                        lhsT=(KTe, KTo)[hh][:, pair, ki * P:(ki + 1) * P],
                        rhs=QTc[:, pair, qsl],
                        start=True, stop=True,
                    )
                ets = []
                for hh, h in enumerate(heads):
                    et = epool.tile([P, 512], BF16, tag="expT")
                    nc.scalar.activation(et[:, :nv], st_ps[hh][:, :nv], AF.Exp,
                                         scale=float(scale))
                    if diag_r >= 0:
                        nc.gpsimd.tensor_tensor(et[:, 0:P], et[:, 0:P], mask[:],
                                                ALU.mult)
                    ets.append(et)
                for hh, h in enumerate(heads):
                    nc.tensor.matmul(
                        ctx_ps[hh][:, c0:512],
                        lhsT=V[:, ki, h, :],
                        rhs=ets[hh][:, :nv],
                        start=(ki == 0), stop=(ki == n_ki - 1),
                    )
                if ki >= ki0:
                    target = -(-nf * (ki + 1 - ki0) // (n_ki - ki0))  # ceil
                    while emitted < target:
                        fillers[emitted]()
                        emitted += 1
            while emitted < nf:
                fillers[emitted]()
                emitted += 1
            return ctx_ps

        def normalize_pair(qc, pair, ctx2):
            """Normalize one head pair's ctx for this q-chunk. psum rows
            64-127 hold the denominator; 1/den = exp(-ln(den)) on the ACT
            engine (both funcs live in one activation table set, see
            _patch_act_tables)."""
            heads = (2 * pair, 2 * pair + 1)
            ldens = []
            for hh in range(2):
                lden = tpool.tile([HD, 512], F32, tag="lden")
                nc.scalar.activation(lden[:], ctx2[hh][HD:2 * HD, :], AF.Ln)
                ldens.append(lden)
            for hh, h in enumerate(heads):
                denb = tpool.tile([HD, 512], F32, tag="denb")
                nc.scalar.activation(denb[:], ldens[hh][:], AF.Exp, scale=-1.0)
                dst = ctxA if h < 2 else ctxB
                nc.vector.tensor_tensor(
                    dst[HD * (h % 2):HD * (h % 2) + HD, qc * 512:qc * 512 + 512],
                    ctx2[hh][0:HD, :], denb[:], ALU.mult)

        # ---- emission (priority) order ----
        # Prologue: chunk 0 projections (serial; overlaps the input DMAs and
        # the warm-up matmuls). Then per chunk: the two attention pair-loops
        # with the previous chunk's out_proj and the next chunk's qk/v
        # projections interleaved as PE fillers; normalize at chunk end.
        uq = qk_units(0, wq, QT)
        uk = qk_units(0, wk, KT)
        uq[0]()
        uq[1]()
        v_unit(0)()
        uk[0]()
        uk[1]()
        v_unit(1)()
        v_unit(2)()
        v_unit(3)()
        # chunk 1's Q and K projections join the prologue: they bridge the
        # PE gap while chunk 0's RoPE + head-rearrange DMAs finish, and
        # chunk 1's rearranges land well before attention(1,0) needs them
        for u in qk_units(1, wq, QT):
            u()
        for u in qk_units(1, wk, KT):
            u()
        for qc in range(NQC):
            if 0 < qc < NQC - 1:
                xt_load(qc + 1)
            f0 = out_units(qc - 1) if qc > 0 else []
            ctx01 = attention_pair(qc, 0, f0, start_frac=0.25)
            normalize_pair(qc, 0, ctx01)
            if qc == 0:
                nc.sync.dma_start(wo[:], wo_d.rearrange("(t p) n -> p t n", p=P))
            f1 = []
            if qc + 1 < NQC:
                if qc > 0:
                    f1 += qk_units(qc + 1, wq, QT)
                    f1 += qk_units(qc + 1, wk, KT)
                f1 += [v_unit(st) for st in range(4 * (qc + 1), 4 * (qc + 1) + 4)]
            ctx23 = attention_pair(qc, 1, f1)
            normalize_pair(qc, 1, ctx23)
        for u in out_units(NQC - 1, evac="scalar"):
            u()


# ----------------------------------------------------------------------------
# host side
# ----------------------------------------------------------------------------

def _rope_tables(s_len):
    pos = np.arange(s_len, dtype=np.float32)
    inv_freq = np.exp(np.arange(0, HD, 2, dtype=np.float32)
                      * (-np.log(10000.0) / HD)).astype(np.float32)
    ang = pos[:, None] * inv_freq[None, :]          # [S, 32]
    sin = np.sin(ang).astype(np.float32)
    cos = np.cos(ang).astype(np.float32)
    # [128, S]: row 32h + i = table for freq i, replicated over the 4 heads
    sinf = np.ascontiguousarray(np.tile(sin.T, (HPC, 1)))
    cosf = np.ascontiguousarray(np.tile(cos.T, (HPC, 1)))
    return sinf, cosf


def _half_perm():
    """Column permutation grouping first/second halves of the 4 heads."""
    first = [64 * h + d for h in range(HPC) for d in range(32)]
    second = [64 * h + d for h in range(HPC) for d in range(32, 64)]
    return np.array(first + second, dtype=np.int64)


def _patch_walrus_flags():
    """Enable walrus' LDWEIGHTS optimization pass (off by default in this
    compile path) so stationary-weight loads pipeline behind matmuls."""
    import concourse.bass_utils as bu
    if getattr(bu.run_command, "_ldw_patched", False):
        return
    orig = bu.run_command

    def run_command(cmd, *a, **kw):
        cmd = [("--enable-ldw-opt=true" if c == "--enable-ldw-opt=false" else c)
               for c in cmd]
        return orig(cmd, *a, **kw)

    run_command._ldw_patched = True
    bu.run_command = run_command


def _patch_act_tables():
    """Make the act-table-load pass map Exp AND Ln to the one table set that
    contains both ('natural_log_exp_and_others'), so the ACT engine loads a
    single table instead of thrashing between 'exp_and_others' and
    'natural_log' at every softmax-denominator reciprocal."""
    import functools
    import concourse.hw_specs as hw_specs
    import concourse.bacc as bacc_mod
    orig = hw_specs.get_activation_tables
    if getattr(orig, "_lnexp_patched", False):
        return
    both = {AF.Exp, AF.Ln}

    @functools.cache
    def patched(arch):
        out = {}
        for name, funcs in orig(arch).items():
            if name == "natural_log_exp_and_others":
                out[name] = set(funcs)
            else:
                out[name] = set(funcs) - both
        return out

    patched._lnexp_patched = True
    hw_specs.get_activation_tables = patched
    bacc_mod.get_activation_tables = patched


def build_program(s_len=S):
    _patch_act_tables()
    if os.environ.get("KERNEL_LDW_OPT", "0") == "1":
        # rejected by walrus codegen for bass-emitted LDWEIGHTS; kept only
        # as an experiment knob
        _patch_walrus_flags()
    nc = bacc.Bacc("TRN2", target_bir_lowering=False, debug=False,
                   num_devices=NCORES)
    io = {
        "xT": nc.dram_tensor("xT", [D, s_len], BF16, kind="ExternalInput").ap(),
        "wq": nc.dram_tensor("wq", [D, JC], BF16, kind="ExternalInput").ap(),
        "wk": nc.dram_tensor("wk", [D, JC], BF16, kind="ExternalInput").ap(),
        "wv": nc.dram_tensor("wv", [D, JC], BF16, kind="ExternalInput").ap(),
        "wo": nc.dram_tensor("wo", [JC, D], BF16, kind="ExternalInput").ap(),
        "sin": nc.dram_tensor("sin", [P, s_len], BF16, kind="ExternalInput").ap(),
        "cos": nc.dram_tensor("cos", [P, s_len], BF16, kind="ExternalInput").ap(),
        "mask": nc.dram_tensor("mask", [P, P], BF16, kind="ExternalInput").ap(),
        "out": nc.dram_tensor("out", [s_len, D], BF16, kind="ExternalOutput").ap(),
    }
    with tile.TileContext(nc) as tc:
        build_core(tc, io, s_len)
    nc.compile()
    return nc


def make_in_maps(x, Wq, Wk, Wv, Wo, s_len=S):
    """Shard the full inputs into one input map per core."""
    perm = _half_perm()
    sinf, cosf = _rope_tables(s_len)
    mask = np.triu(np.ones((P, P), dtype=np.float32)).astype(ml_dtypes.bfloat16)
    bf = ml_dtypes.bfloat16
    in_maps = []
    for c in range(NCORES):
        b, g = divmod(c, NCORES // B)
        cols = slice(JC * g, JC * (g + 1))
        in_maps.append({
            "xT": np.ascontiguousarray(x[b].T).astype(bf),
            "wq": np.ascontiguousarray(Wq[:, cols][:, perm]).astype(bf),
            "wk": np.ascontiguousarray(Wk[:, cols][:, perm]).astype(bf),
            "wv": np.ascontiguousarray(Wv[:, cols]).astype(bf),
            "wo": np.ascontiguousarray(Wo[cols, :]).astype(bf),
            "sin": sinf.astype(bf), "cos": cosf.astype(bf), "mask": mask,
        })
    return in_maps


_CACHED_NC = None


def kernel(x, Wq, bq, Wk, bk, Wv, bv, Wo, bo, **run_kwargs):
    global _CACHED_NC
    x, Wq, bq, Wk, bk, Wv, bv, Wo, bo = (
        np.asarray(a, dtype=np.float32)
        for a in (x, Wq, bq, Wk, bk, Wv, bv, Wo, bo))
    assert not (np.any(bq) or np.any(bk) or np.any(bv)), \
        "nonzero qkv biases not supported by this build"
    if _CACHED_NC is None:
        _CACHED_NC = build_program(S)
    in_maps = make_in_maps(x, Wq, Wk, Wv, Wo, S)
    res = run_bass_kernel_spmd(_CACHED_NC, in_maps, list(range(NCORES)),
                               **run_kwargs)
    out = np.zeros((B, S, D), dtype=np.float32)
    for c in range(NCORES):
        b = c // (NCORES // B)
        out[b] += res.results[c]["out"].astype(np.float32)
    out += bo[None, None, :]
    if run_kwargs:
        kernel.last_result = res
    return out
